# revision 8
# baseline (speedup 1.0000x reference)
"""Causal single-head attention on 8 Trainium2 NeuronCores.

Problem: x [4, 2048, 1024], w_q/w_k/w_v [1024, 1024] (nn.Linear convention,
y = x @ W.T). Computes q,k,v projections, causal softmax(q k^T / sqrt(D)) @ v.

Sharding: 2 cores per batch element; query tiles interleaved by parity so
every core has one 128-query tile per kv-length class k (window 256*k).

v3 design (vs the fp32r baseline):
  * K/V projections are computed for HALF the tokens per core (the pair
    core computes the other half) and exchanged with an intra-pair
    AllGather (bf16 payload) — removes the duplicated half of the K/V
    GEMMs (-28% PE work). K and V share the same x tiles (x loaded once).
  * Everything runs in bf16 on the PE (same rate as fp32r, half the DMA
    bytes, much cheaper LDWEIGHTS). fp8 was measured at rel err 6e-2
    (> 2e-2 gate) and rejected; all-bf16 measures ~5e-3.
  * Q^T stays in SBUF (no DRAM spill).
  * PSUM->SBUF P^T copies alternate between ACT and DVE (the slot phase
    is vector-throughput-sensitive).
"""
import numpy as np
from contextlib import ExitStack

import concourse.bass as bass
import concourse.tile as tile
import concourse.mybir as mybir
from concourse.bass_utils import run_bass_kernel_spmd
from concourse.masks import make_identity

F32 = mybir.dt.float32
BF16 = mybir.dt.bfloat16
AF = mybir.ActivationFunctionType
AX = mybir.AxisListType

B, S, E, D = 4, 2048, 1024, 1024
NCORES = 8
NSLOT = 8              # slots k=1..8, kv window = 256*k tokens
NQ = NSLOT * 128       # queries per core
EC = E // 128          # e-chunks
DC = D // 128          # d-chunks
HT = S // 2            # tokens per core half
SCALE = 1.0 / 32.0     # 1/sqrt(D)
MASKVAL = -30000.0

_prog = None


def _split_multi_waits(nc, max_waits=1):
    """The walrus build in this container has one sync-wait slot per
    instruction; hoist extra waits onto preceding same-engine NoOps."""
    n = 0
    for f in nc.m.functions:
        for b in f.blocks:
            insts = b.instructions
            out = []
            changed = False
            for ins in insts:
                si = ins.sync_info
                if si is not None and len(si.on_wait) > max_waits:
                    waits = list(si.on_wait)
                    for w in waits[:-max_waits]:
                        nop = mybir.InstNoOp(name=f"I-waitsplit-{n}")
                        n += 1
                        nop.engine = ins.engine
                        nop.sync_info = mybir.SyncInfo(on_wait=[w], on_update=[])
                        out.append(nop)
                    ins.sync_info = mybir.SyncInfo(
                        on_wait=waits[-max_waits:], on_update=list(si.on_update))
                    changed = True
                out.append(ins)
            if changed:
                b.instructions = out
    return nc


def _build(split=True):
    nc = bass.Bass(trn_type="TRN2", target_bir_lowering=False, debug=False)
    xkvT = nc.dram_tensor("xkvT", [E, HT], BF16, kind="ExternalInput").ap()
    xqT = nc.dram_tensor("xqT", [E, NQ], BF16, kind="ExternalInput").ap()
    wqT = nc.dram_tensor("wqT", [E, D], BF16, kind="ExternalInput").ap()
    wkT = nc.dram_tensor("wkT", [E, D], BF16, kind="ExternalInput").ap()
    wvT = nc.dram_tensor("wvT", [E, D], BF16, kind="ExternalInput").ap()
    maskin = nc.dram_tensor("mask", [128, 256], F32, kind="ExternalInput").ap()
    out = nc.dram_tensor("out", [NQ, D], F32, kind="ExternalOutput").ap()
    # AllGather bounce buffers: my K^T half [dc, 128, HT], my V half
    # [tc, 128, D]; gathered = [member, ...] in absolute token order.
    bK = nc.dram_tensor("bK", [DC, 128, HT], BF16).ap()
    bV = nc.dram_tensor("bV", [HT // 256, 128, D], BF16).ap()
    gK = nc.dram_tensor("gK", [2, DC, 128, HT], BF16).ap()
    bV2 = nc.dram_tensor("bV2", [HT // 256, 128, D], BF16).ap()
    gV2 = nc.dram_tensor("gV2", [2, HT // 256, 128, D], BF16).ap()
    gV = nc.dram_tensor("gV", [2, HT // 256, 128, D], BF16).ap()
    bW = nc.dram_tensor("bW", [128, 4], BF16).ap()
    gW = nc.dram_tensor("gW", [2, 128, 4], BF16).ap()
    RG = [[0, 1], [2, 3], [4, 5], [6, 7]]

    with tile.TileContext(nc) as tc, ExitStack() as ctx:
        # tiny warmup AllGather: absorbs the one-time CC stream startup
        # (~25us) under the input-DMA phase so the real gathers start
        # transferring immediately
        nc.gpsimd.collective_compute(
            "AllGather", mybir.AluOpType.bypass, replica_groups=RG,
            ins=[bW[:]], outs=[gW[:]])
        const = ctx.enter_context(tc.tile_pool(name="const", bufs=1))
        ident = const.tile([128, 128], BF16)
        make_identity(nc, ident[:])
        mask_sb = const.tile([128, 256], F32)
        nc.sync.dma_start(mask_sb[:], maskin[:])

        # persistent attention operands (bf16)
        kvq = ctx.enter_context(tc.tile_pool(name="kvq", bufs=1))
        kts = kvq.tile([128, DC, S], BF16)        # [d-in-chunk, dc, t]
        vts = kvq.tile([128, S // 128, D], BF16)  # [t-in-chunk, tc, d]
        qts = kvq.tile([128, DC, NQ], BF16)       # [d-in-chunk, dc, q]

        # ---- Phase 1: K^T half + V half (shared x tiles), then Q^T ----
        with tc.tile_pool(name="wp", bufs=1) as wp, \
             tc.tile_pool(name="xp", bufs=1) as xp, \
             tc.tile_pool(name="st", bufs=1) as stp, \
             tc.tile_pool(name="pp1", bufs=4, space="PSUM") as pp:
            wk = [wp.tile([128, D], BF16, name=f"wk{e}") for e in range(EC)]
            wv = [wp.tile([128, D], BF16, name=f"wv{e}") for e in range(EC)]
            wq = [wp.tile([128, D], BF16, name=f"wq{e}") for e in range(EC)]
            xgs = [[xp.tile([128, 512], BF16, name=f"x1_{g}_{e}")
                    for e in range(EC)] for g in range(HT // 512)]
            # critical first wave, quarter-split so the first (wk, x)
            # chunks land on parallel DMA queues fast
            for e in range(EC):
                nc.sync.dma_start(wk[e][:, :256],
                                  wkT[e * 128:(e + 1) * 128, :256])
                nc.sync.dma_start(wk[e][:, 256:512],
                                  wkT[e * 128:(e + 1) * 128, 256:512])
                nc.sync.dma_start(xgs[0][e][:, :256],
                                  xkvT[e * 128:(e + 1) * 128, :256])
                nc.sync.dma_start(xgs[0][e][:, 256:],
                                  xkvT[e * 128:(e + 1) * 128, 256:512])
            for e in range(EC):
                nc.sync.dma_start(wk[e][:, 512:],
                                  wkT[e * 128:(e + 1) * 128, 512:])
                nc.sync.dma_start(xgs[1][e][:], xkvT[e * 128:(e + 1) * 128,
                                                     512:1024])
            for e in range(EC):
                nc.sync.dma_start(wv[e][:], wvT[e * 128:(e + 1) * 128, :])
            for e in range(EC):
                nc.sync.dma_start(wq[e][:], wqT[e * 128:(e + 1) * 128, :])
            # K^T: both groups, then AllGather K immediately
            for g in range(HT // 512):
                xg = xgs[g]
                if g == 0:
                    # e-outer startup: each arriving (wk,x) chunk feeds 4
                    # matmuls immediately
                    for dh in range(2):
                        psl = [pp.tile([128, 512], F32, name=f"pk0_{dh}_{d}",
                                       tag="pp") for d in range(4)]
                        for e in range(EC):
                            for d in range(4):
                                dd = dh * 4 + d
                                nc.tensor.matmul(psl[d][:],
                                                 wk[e][:, dd * 128:(dd + 1) * 128],
                                                 xg[e][:], start=(e == 0),
                                                 stop=(e == EC - 1))
                        for d in range(4):
                            dd = dh * 4 + d
                            ks = stp.tile([128, 512], BF16, name="ks", tag="ks",
                                          bufs=4)
                            nc.vector.tensor_copy(ks[:], psl[d][:])
                            nc.sync.dma_start(bK[dd, :, :512], ks[:])
                else:
                    for d in range(DC):
                        ps = pp.tile([128, 512], F32, name=f"pk{g}_{d}", tag="pp")
                        for e in range(EC):
                            nc.tensor.matmul(ps[:], wk[e][:, d * 128:(d + 1) * 128],
                                             xg[e][:], start=(e == 0),
                                             stop=(e == EC - 1))
                        ks = stp.tile([128, 512], BF16, name="ks", tag="ks", bufs=4)
                        nc.vector.tensor_copy(ks[:], ps[:])
                        nc.sync.dma_start(bK[d, :, g * 512:(g + 1) * 512], ks[:])
            nc.gpsimd.collective_compute(
                "AllGather", mybir.AluOpType.bypass, replica_groups=RG,
                ins=[bK[:]], outs=[gK[:]])
            for m in range(2):
                nc.sync.dma_start(
                    kts[:, :, m * HT:(m + 1) * HT],
                    gK[m].rearrange("c p t -> p c t"))
            # V: both groups (same x tiles), AllGather per 512-token group
            for g in range(HT // 512):
                xg = xgs[g]
                bVg = bV if g == 0 else bV2
                for st in range(4):
                    for dh in range(2):
                        ps = pp.tile([128, 512], F32, name=f"pv{g}_{st}_{dh}",
                                     tag="pp")
                        for e in range(EC):
                            nc.tensor.matmul(ps[:], xg[e][:, st * 128:(st + 1) * 128],
                                             wv[e][:, dh * 512:(dh + 1) * 512],
                                             start=(e == 0), stop=(e == EC - 1))
                        vs = stp.tile([128, 512], BF16, name="vs", tag="vs", bufs=4)
                        nc.vector.tensor_copy(vs[:], ps[:])
                        nc.sync.dma_start(bVg[st, :, dh * 512:(dh + 1) * 512],
                                          vs[:])
                gVg = gV if g == 0 else gV2
                nc.gpsimd.collective_compute(
                    "AllGather", mybir.AluOpType.bypass, replica_groups=RG,
                    ins=[bVg[:, :, :] if g == 0 else bV2[:]], outs=[gVg[:]])
                for m in range(2):
                    # member m, group g covers absolute t-chunks
                    # m*8 + g*4 .. +4
                    nc.sync.dma_start(
                        vts[:, m * 8 + g * 4:m * 8 + g * 4 + 4, :],
                        gVg[m].rearrange("c p d -> p c d"))

            # Q^T straight into SBUF (no DRAM spill)
            for g in range(NQ // 512):
                xq = [xp.tile([128, 512], BF16, name=f"x1q_{e}", tag=f"xg{e}",
                              bufs=2) for e in range(EC)]
                for e in range(EC):
                    nc.sync.dma_start(xq[e][:], xqT[e * 128:(e + 1) * 128,
                                                    g * 512:(g + 1) * 512])
                for d in range(DC):
                    ps = pp.tile([128, 512], F32, name=f"pq{g}_{d}", tag="pp")
                    for e in range(EC):
                        nc.tensor.matmul(ps[:], wq[e][:, d * 128:(d + 1) * 128],
                                         xq[e][:], start=(e == 0),
                                         stop=(e == EC - 1))
                    nc.scalar.copy(qts[:, d, g * 512:(g + 1) * 512], ps[:])

        # ---- Phase 2: attention, one slot per kv-length class ----
        # big/small interleave keeps PE fed with the small slot's matmuls
        # while the big slot's softmax chain runs on ACT/DVE
        slot_order = [4, 1, 3, 2, 8, 5, 7, 6]
        with tc.tile_pool(name="att", bufs=1) as ap_, \
             tc.tile_pool(name="ps3", bufs=1, space="PSUM") as pp3:
            for k in slot_order:
                kv = 256 * k
                nch = kv // 128
                ngr = (kv + 511) // 512
                q0 = (k - 1) * 128

                s_ps = [pp3.tile([128, 512], F32, name=f"sps{k}_{g}", tag="sps",
                                 bufs=4) for g in range(ngr)]
                for d in range(DC):
                    lhs = qts[:, d, q0:q0 + 128]
                    for g in range(ngr):
                        w = min(512, kv - g * 512)
                        nc.tensor.matmul(s_ps[g][:, :w], lhs,
                                         kts[:, d, g * 512:g * 512 + w],
                                         start=(d == 0), stop=(d == DC - 1))

                # psum -> sbuf copies (mask folded into the last 256 cols)
                # with per-group running max
                s_sb = ap_.tile([128, 2048], F32, name=f"s{k}", tag="s", bufs=2)
                mparts = ap_.tile([128, 4], F32, name=f"mp{k}", tag="mp", bufs=2)
                lg = ngr - 1
                lw = kv - lg * 512
                for g in range(lg):
                    nc.scalar.copy(s_sb[:, g * 512:(g + 1) * 512], s_ps[g][:])
                if lw == 512:
                    nc.scalar.copy(s_sb[:, kv - 512:kv - 256], s_ps[lg][:, :256])
                    nc.vector.tensor_add(s_sb[:, kv - 256:kv],
                                         s_ps[lg][:, 256:512], mask_sb[:])
                else:
                    nc.vector.tensor_add(s_sb[:, kv - 256:kv],
                                         s_ps[lg][:, :256], mask_sb[:])
                for g in range(ngr):
                    w = min(512, kv - g * 512)
                    nc.vector.reduce_max(mparts[:, g:g + 1],
                                         s_sb[:, g * 512:g * 512 + w], axis=AX.X)

                m = ap_.tile([128, 1], F32, name=f"m{k}", tag="m", bufs=2)
                nc.vector.reduce_max(m[:], mparts[:, :ngr], axis=AX.X)
                negm = ap_.tile([128, 1], F32, name=f"negm{k}", tag="negm", bufs=2)
                nc.scalar.mul(negm[:], m[:], -SCALE)
                # per-group exp so transposes can start before the whole row
                # is exponentiated; per-group sums summed at the end
                p_sb = ap_.tile([128, 2048], BF16, name=f"p{k}", tag="p", bufs=2)
                lparts = ap_.tile([128, 4], F32, name=f"lp{k}", tag="lp", bufs=2)
                for g in range(ngr):
                    w = min(512, kv - g * 512)
                    nc.scalar.activation(p_sb[:, g * 512:g * 512 + w],
                                         s_sb[:, g * 512:g * 512 + w], AF.Exp,
                                         bias=negm[:], scale=SCALE,
                                         accum_out=lparts[:, g:g + 1])
                lsum = ap_.tile([128, 1], F32, name=f"lsum{k}", tag="lsum", bufs=2)
                nc.vector.reduce_sum(lsum[:], lparts[:, :ngr], axis=AX.X)
                linv = ap_.tile([128, 1], F32, name=f"linv{k}", tag="linv", bufs=2)
                nc.vector.reciprocal(linv[:], lsum[:])

                # P^T via PE transpose; copies split across ACT/DVE
                pt = ap_.tile([128, S // 128, 128], BF16, name=f"pt{k}", tag="pt",
                              bufs=2)
                for c in range(nch):
                    tps = pp3.tile([128, 128], BF16, name=f"tp{k}_{c}", tag="tps",
                                   bufs=2)
                    nc.tensor.transpose(tps[:], p_sb[:, c * 128:(c + 1) * 128],
                                        ident[:])
                    if c % 2 == 0:
                        nc.scalar.copy(pt[:, c, :], tps[:])
                    else:
                        nc.vector.tensor_copy(pt[:, c, :], tps[:])

                o_ps = [pp3.tile([128, 512], F32, name=f"op{k}_{h}", tag="ops",
                                 bufs=2) for h in range(2)]
                for c in range(nch):
                    lhs = pt[:, c, :]
                    for h in range(2):
                        nc.tensor.matmul(o_ps[h][:], lhs,
                                         vts[:, c, h * 512:(h + 1) * 512],
                                         start=(c == 0), stop=(c == nch - 1))

                o_sb = ap_.tile([128, D], F32, name=f"o{k}", tag="o", bufs=2)
                for h in range(2):
                    nc.vector.tensor_scalar_mul(o_sb[:, h * 512:(h + 1) * 512],
                                                o_ps[h][:], linv[:])
                nc.sync.dma_start(out[(k - 1) * 128:k * 128, :512],
                                  o_sb[:, :512])
                nc.sync.dma_start(out[(k - 1) * 128:k * 128, 512:],
                                  o_sb[:, 512:])
    if split:
        _split_multi_waits(nc)
    return nc


def _masks():
    j = np.arange(256)[None, :]
    i = np.arange(128)[:, None]
    mask0 = np.where(j <= i, 0.0, MASKVAL).astype(np.float32)
    mask1 = np.where(j <= 128 + i, 0.0, MASKVAL).astype(np.float32)
    return mask0, mask1


def _in_maps(x, w_q, w_k, w_v):
    import ml_dtypes
    bf = ml_dtypes.bfloat16
    x = np.asarray(x, dtype=np.float32)
    wqT = np.ascontiguousarray(np.asarray(w_q, np.float32).T).astype(bf)
    wkT = np.ascontiguousarray(np.asarray(w_k, np.float32).T).astype(bf)
    wvT = np.ascontiguousarray(np.asarray(w_v, np.float32).T).astype(bf)
    mask0, mask1 = _masks()

    in_maps = []
    for c in range(NCORES):
        b, p = divmod(c, 2)
        xb = x[b]                                    # [S, E]
        xkvT = np.ascontiguousarray(xb[p * HT:(p + 1) * HT].T).astype(bf)
        qrows = np.concatenate(
            [xb[128 * (2 * (k - 1) + p):128 * (2 * (k - 1) + p) + 128, :]
             for k in range(1, NSLOT + 1)], axis=0)  # [NQ, E]
        xqT = np.ascontiguousarray(qrows.T).astype(bf)
        in_maps.append({
            "xkvT": xkvT, "xqT": xqT,
            "wqT": wqT, "wkT": wkT, "wvT": wvT,
            "mask": mask0 if p == 0 else mask1,
        })
    return in_maps


def _scatter(per_core_out):
    out = np.empty((B, S, D), dtype=np.float32)
    for c in range(NCORES):
        b, p = divmod(c, 2)
        oc = per_core_out[c]                         # [NQ, D]
        for k in range(1, NSLOT + 1):
            g = 2 * (k - 1) + p
            out[b, 128 * g:128 * (g + 1), :] = oc[128 * (k - 1):128 * k, :]
    return out


def kernel(x, w_q, w_k, w_v):
    global _prog
    if _prog is None:
        _prog = _build()
    in_maps = _in_maps(x, w_q, w_k, w_v)
    res = run_bass_kernel_spmd(_prog, in_maps, list(range(NCORES)))
    return _scatter([res.results[c]["out"] for c in range(NCORES)])


# revision 9
# speedup vs baseline: 1.0718x; 1.0718x over previous
"""Causal single-head attention on 8 Trainium2 NeuronCores.

Problem: x [4, 2048, 1024], w_q/w_k/w_v [1024, 1024] (nn.Linear convention,
y = x @ W.T). Computes q,k,v projections, causal softmax(q k^T / sqrt(D)) @ v.

Sharding: 2 cores per batch element; query tiles interleaved by parity so
every core has one 128-query tile per kv-length class k (window 256*k).

v3 design (vs the fp32r baseline):
  * K/V projections are computed for HALF the tokens per core (the pair
    core computes the other half) and exchanged with an intra-pair
    AllGather (bf16 payload) — removes the duplicated half of the K/V
    GEMMs (-28% PE work). K and V share the same x tiles (x loaded once).
  * Everything runs in bf16 on the PE (same rate as fp32r, half the DMA
    bytes, much cheaper LDWEIGHTS). fp8 was measured at rel err 6e-2
    (> 2e-2 gate) and rejected; all-bf16 measures ~5e-3.
  * Q^T stays in SBUF (no DRAM spill).
  * PSUM->SBUF P^T copies alternate between ACT and DVE (the slot phase
    is vector-throughput-sensitive).
"""
import numpy as np
from contextlib import ExitStack

import concourse.bass as bass
import concourse.tile as tile
import concourse.mybir as mybir
from concourse.bass_utils import run_bass_kernel_spmd
from concourse.masks import make_identity

F32 = mybir.dt.float32
BF16 = mybir.dt.bfloat16
AF = mybir.ActivationFunctionType
AX = mybir.AxisListType

B, S, E, D = 4, 2048, 1024, 1024
NCORES = 8
NSLOT = 8              # slots k=1..8, kv window = 256*k tokens
NQ = NSLOT * 128       # queries per core
EC = E // 128          # e-chunks
DC = D // 128          # d-chunks
HT = S // 2            # tokens per core half
SCALE = 1.0 / 32.0     # 1/sqrt(D)
MASKVAL = -30000.0

_prog = None


def _split_multi_waits(nc, max_waits=1):
    """The walrus build in this container has one sync-wait slot per
    instruction; hoist extra waits onto preceding same-engine NoOps."""
    n = 0
    for f in nc.m.functions:
        for b in f.blocks:
            insts = b.instructions
            out = []
            changed = False
            for ins in insts:
                si = ins.sync_info
                if si is not None and len(si.on_wait) > max_waits:
                    waits = list(si.on_wait)
                    for w in waits[:-max_waits]:
                        nop = mybir.InstNoOp(name=f"I-waitsplit-{n}")
                        n += 1
                        nop.engine = ins.engine
                        nop.sync_info = mybir.SyncInfo(on_wait=[w], on_update=[])
                        out.append(nop)
                    ins.sync_info = mybir.SyncInfo(
                        on_wait=waits[-max_waits:], on_update=list(si.on_update))
                    changed = True
                out.append(ins)
            if changed:
                b.instructions = out
    return nc


def _build(split=True):
    nc = bass.Bass(trn_type="TRN2", target_bir_lowering=False, debug=False)
    xkvT = nc.dram_tensor("xkvT", [E, HT], BF16, kind="ExternalInput").ap()
    xqT = nc.dram_tensor("xqT", [E, NQ], BF16, kind="ExternalInput").ap()
    wqT = nc.dram_tensor("wqT", [E, D], BF16, kind="ExternalInput").ap()
    wkT = nc.dram_tensor("wkT", [E, D], BF16, kind="ExternalInput").ap()
    wvT = nc.dram_tensor("wvT", [E, D], BF16, kind="ExternalInput").ap()
    maskin = nc.dram_tensor("mask", [128, 256], F32, kind="ExternalInput").ap()
    out = nc.dram_tensor("out", [NQ, D], F32, kind="ExternalOutput").ap()
    # AllGather bounce buffers: my K^T half [dc, 128, HT], my V half
    # [tc, 128, D]; gathered = [member, ...] in absolute token order.
    bK = nc.dram_tensor("bK", [DC, 128, HT], BF16).ap()
    bV = nc.dram_tensor("bV", [HT // 256, 128, D], BF16).ap()
    gK = nc.dram_tensor("gK", [2, DC, 128, HT], BF16).ap()
    bV2 = nc.dram_tensor("bV2", [HT // 256, 128, D], BF16).ap()
    gV2 = nc.dram_tensor("gV2", [2, HT // 256, 128, D], BF16).ap()
    gV = nc.dram_tensor("gV", [2, HT // 256, 128, D], BF16).ap()
    bW = nc.dram_tensor("bW", [128, 4], BF16).ap()
    gW = nc.dram_tensor("gW", [2, 128, 4], BF16).ap()
    RG = [[0, 1], [2, 3], [4, 5], [6, 7]]

    with tile.TileContext(nc) as tc, ExitStack() as ctx:
        # tiny warmup AllGather: absorbs the one-time CC stream startup
        # (~25us) under the input-DMA phase so the real gathers start
        # transferring immediately
        nc.gpsimd.collective_compute(
            "AllGather", mybir.AluOpType.bypass, replica_groups=RG,
            ins=[bW[:]], outs=[gW[:]])
        const = ctx.enter_context(tc.tile_pool(name="const", bufs=1))
        ident = const.tile([128, 128], BF16)
        make_identity(nc, ident[:])
        mask_sb = const.tile([128, 256], F32)
        nc.sync.dma_start(mask_sb[:], maskin[:])

        # persistent attention operands (bf16)
        kvq = ctx.enter_context(tc.tile_pool(name="kvq", bufs=1))
        kts = kvq.tile([128, DC, S], BF16)        # [d-in-chunk, dc, t]
        vts = kvq.tile([128, S // 128, D], BF16)  # [t-in-chunk, tc, d]
        qts = kvq.tile([128, DC, NQ], BF16)       # [d-in-chunk, dc, q]

        # ---- Phase 1: K^T half + V half (shared x tiles), then Q^T ----
        with tc.tile_pool(name="wp", bufs=1) as wp, \
             tc.tile_pool(name="xp", bufs=1) as xp, \
             tc.tile_pool(name="st", bufs=1) as stp, \
             tc.tile_pool(name="pp1", bufs=4, space="PSUM") as pp:
            wk = [wp.tile([128, D], BF16, name=f"wk{e}") for e in range(EC)]
            wv = [wp.tile([128, D], BF16, name=f"wv{e}") for e in range(EC)]
            wq = [wp.tile([128, D], BF16, name=f"wq{e}") for e in range(EC)]
            xgs = [[xp.tile([128, 512], BF16, name=f"x1_{g}_{e}")
                    for e in range(EC)] for g in range(HT // 512)]
            # critical first wave, quarter-split so the first (wk, x)
            # chunks land on parallel DMA queues fast
            for e in range(EC):
                nc.sync.dma_start(wk[e][:, :256],
                                  wkT[e * 128:(e + 1) * 128, :256])
                nc.sync.dma_start(wk[e][:, 256:512],
                                  wkT[e * 128:(e + 1) * 128, 256:512])
                nc.sync.dma_start(xgs[0][e][:, :256],
                                  xkvT[e * 128:(e + 1) * 128, :256])
                nc.sync.dma_start(xgs[0][e][:, 256:],
                                  xkvT[e * 128:(e + 1) * 128, 256:512])
            for e in range(EC):
                nc.sync.dma_start(wk[e][:, 512:],
                                  wkT[e * 128:(e + 1) * 128, 512:])
                nc.sync.dma_start(wv[e][:], wvT[e * 128:(e + 1) * 128, :])
            for e in range(EC):
                nc.sync.dma_start(xgs[1][e][:], xkvT[e * 128:(e + 1) * 128,
                                                     512:1024])
                nc.sync.dma_start(wq[e][:], wqT[e * 128:(e + 1) * 128, :])
            # K and V interleaved per 512-token group (keeps PE/DVE/PSUM
            # pipelined); V AllGather fires per group, K's after group 1
            for g in range(HT // 512):
                xg = xgs[g]
                if g == 0:
                    # e-outer startup: each arriving (wk,x) chunk feeds 4
                    # matmuls immediately
                    for dh in range(2):
                        psl = [pp.tile([128, 512], F32, name=f"pk0_{dh}_{d}",
                                       tag="pp") for d in range(4)]
                        for e in range(EC):
                            for d in range(4):
                                dd = dh * 4 + d
                                nc.tensor.matmul(psl[d][:],
                                                 wk[e][:, dd * 128:(dd + 1) * 128],
                                                 xg[e][:], start=(e == 0),
                                                 stop=(e == EC - 1))
                        for d in range(4):
                            dd = dh * 4 + d
                            ks = stp.tile([128, 512], BF16, name="ks", tag="ks",
                                          bufs=4)
                            nc.vector.tensor_copy(ks[:], psl[d][:])
                            nc.sync.dma_start(bK[dd, :, :512], ks[:])
                else:
                    for d in range(DC):
                        ps = pp.tile([128, 512], F32, name=f"pk{g}_{d}", tag="pp")
                        for e in range(EC):
                            nc.tensor.matmul(ps[:], wk[e][:, d * 128:(d + 1) * 128],
                                             xg[e][:], start=(e == 0),
                                             stop=(e == EC - 1))
                        ks = stp.tile([128, 512], BF16, name="ks", tag="ks", bufs=4)
                        nc.vector.tensor_copy(ks[:], ps[:])
                        nc.sync.dma_start(bK[d, :, g * 512:(g + 1) * 512], ks[:])
                # V for this group (same xg tiles; stationary x t-chunks)
                bVg = bV if g == 0 else bV2
                for st in range(4):
                    for dh in range(2):
                        ps = pp.tile([128, 512], F32, name=f"pv{g}_{st}_{dh}",
                                     tag="pp")
                        for e in range(EC):
                            nc.tensor.matmul(ps[:], xg[e][:, st * 128:(st + 1) * 128],
                                             wv[e][:, dh * 512:(dh + 1) * 512],
                                             start=(e == 0), stop=(e == EC - 1))
                        vs = stp.tile([128, 512], BF16, name="vs", tag="vs", bufs=4)
                        nc.vector.tensor_copy(vs[:], ps[:])
                        nc.sync.dma_start(bVg[st, :, dh * 512:(dh + 1) * 512],
                                          vs[:])
                if g == HT // 512 - 1:
                    # K complete: gather it (CC stream is warm) before V g1
                    nc.gpsimd.collective_compute(
                        "AllGather", mybir.AluOpType.bypass, replica_groups=RG,
                        ins=[bK[:]], outs=[gK[:]])
                    for m in range(2):
                        nc.sync.dma_start(
                            kts[:, :, m * HT:(m + 1) * HT],
                            gK[m].rearrange("c p t -> p c t"))
                gVg = gV if g == 0 else gV2
                nc.gpsimd.collective_compute(
                    "AllGather", mybir.AluOpType.bypass, replica_groups=RG,
                    ins=[bVg[:]], outs=[gVg[:]])
                for m in range(2):
                    nc.sync.dma_start(
                        vts[:, m * 8 + g * 4:m * 8 + g * 4 + 4, :],
                        gVg[m].rearrange("c p d -> p c d"))

            # Q^T straight into SBUF (no DRAM spill)
            for g in range(NQ // 512):
                xq = [xp.tile([128, 512], BF16, name=f"x1q_{e}", tag=f"xg{e}",
                              bufs=2) for e in range(EC)]
                for e in range(EC):
                    nc.sync.dma_start(xq[e][:], xqT[e * 128:(e + 1) * 128,
                                                    g * 512:(g + 1) * 512])
                for d in range(DC):
                    ps = pp.tile([128, 512], F32, name=f"pq{g}_{d}", tag="pp")
                    for e in range(EC):
                        nc.tensor.matmul(ps[:], wq[e][:, d * 128:(d + 1) * 128],
                                         xq[e][:], start=(e == 0),
                                         stop=(e == EC - 1))
                    nc.scalar.copy(qts[:, d, g * 512:(g + 1) * 512], ps[:])

        # ---- Phase 2: attention, one slot per kv-length class ----
        # big/small interleave keeps PE fed with the small slot's matmuls
        # while the big slot's softmax chain runs on ACT/DVE
        slot_order = [8, 3, 7, 4, 6, 5, 2, 1]
        with tc.tile_pool(name="att", bufs=1) as ap_, \
             tc.tile_pool(name="ps3", bufs=1, space="PSUM") as pp3:
            for k in slot_order:
                kv = 256 * k
                nch = kv // 128
                ngr = (kv + 511) // 512
                q0 = (k - 1) * 128

                s_ps = [pp3.tile([128, 512], F32, name=f"sps{k}_{g}", tag="sps",
                                 bufs=4) for g in range(ngr)]
                for d in range(DC):
                    lhs = qts[:, d, q0:q0 + 128]
                    for g in range(ngr):
                        w = min(512, kv - g * 512)
                        nc.tensor.matmul(s_ps[g][:, :w], lhs,
                                         kts[:, d, g * 512:g * 512 + w],
                                         start=(d == 0), stop=(d == DC - 1))

                # psum -> sbuf copies (mask folded into the last 256 cols)
                # with per-group running max
                s_sb = ap_.tile([128, 2048], F32, name=f"s{k}", tag="s", bufs=2)
                mparts = ap_.tile([128, 4], F32, name=f"mp{k}", tag="mp", bufs=2)
                lg = ngr - 1
                lw = kv - lg * 512
                for g in range(lg):
                    nc.scalar.copy(s_sb[:, g * 512:(g + 1) * 512], s_ps[g][:])
                if lw == 512:
                    nc.scalar.copy(s_sb[:, kv - 512:kv - 256], s_ps[lg][:, :256])
                    nc.vector.tensor_add(s_sb[:, kv - 256:kv],
                                         s_ps[lg][:, 256:512], mask_sb[:])
                else:
                    nc.vector.tensor_add(s_sb[:, kv - 256:kv],
                                         s_ps[lg][:, :256], mask_sb[:])
                for g in range(ngr):
                    w = min(512, kv - g * 512)
                    nc.vector.reduce_max(mparts[:, g:g + 1],
                                         s_sb[:, g * 512:g * 512 + w], axis=AX.X)

                m = ap_.tile([128, 1], F32, name=f"m{k}", tag="m", bufs=2)
                nc.vector.reduce_max(m[:], mparts[:, :ngr], axis=AX.X)
                negm = ap_.tile([128, 1], F32, name=f"negm{k}", tag="negm", bufs=2)
                nc.scalar.mul(negm[:], m[:], -SCALE)
                # per-group exp so transposes can start before the whole row
                # is exponentiated; per-group sums summed at the end
                p_sb = ap_.tile([128, 2048], BF16, name=f"p{k}", tag="p", bufs=2)
                lparts = ap_.tile([128, 4], F32, name=f"lp{k}", tag="lp", bufs=2)
                for g in range(ngr):
                    w = min(512, kv - g * 512)
                    nc.scalar.activation(p_sb[:, g * 512:g * 512 + w],
                                         s_sb[:, g * 512:g * 512 + w], AF.Exp,
                                         bias=negm[:], scale=SCALE,
                                         accum_out=lparts[:, g:g + 1])
                lsum = ap_.tile([128, 1], F32, name=f"lsum{k}", tag="lsum", bufs=2)
                nc.vector.reduce_sum(lsum[:], lparts[:, :ngr], axis=AX.X)
                linv = ap_.tile([128, 1], F32, name=f"linv{k}", tag="linv", bufs=2)
                nc.vector.reciprocal(linv[:], lsum[:])

                # P^T via PE transpose; copies split across ACT/DVE
                pt = ap_.tile([128, S // 128, 128], BF16, name=f"pt{k}", tag="pt",
                              bufs=2)
                for c in range(nch):
                    tps = pp3.tile([128, 128], BF16, name=f"tp{k}_{c}", tag="tps",
                                   bufs=2)
                    nc.tensor.transpose(tps[:], p_sb[:, c * 128:(c + 1) * 128],
                                        ident[:])
                    if c % 2 == 0:
                        nc.scalar.copy(pt[:, c, :], tps[:])
                    else:
                        nc.vector.tensor_copy(pt[:, c, :], tps[:])

                o_ps = [pp3.tile([128, 512], F32, name=f"op{k}_{h}", tag="ops",
                                 bufs=2) for h in range(2)]
                for c in range(nch):
                    lhs = pt[:, c, :]
                    for h in range(2):
                        nc.tensor.matmul(o_ps[h][:], lhs,
                                         vts[:, c, h * 512:(h + 1) * 512],
                                         start=(c == 0), stop=(c == nch - 1))

                o_sb = ap_.tile([128, D], F32, name=f"o{k}", tag="o", bufs=2)
                for h in range(2):
                    nc.vector.tensor_scalar_mul(o_sb[:, h * 512:(h + 1) * 512],
                                                o_ps[h][:], linv[:])
                nc.sync.dma_start(out[(k - 1) * 128:k * 128, :512],
                                  o_sb[:, :512])
                nc.sync.dma_start(out[(k - 1) * 128:k * 128, 512:],
                                  o_sb[:, 512:])
    if split:
        _split_multi_waits(nc)
    return nc


def _masks():
    j = np.arange(256)[None, :]
    i = np.arange(128)[:, None]
    mask0 = np.where(j <= i, 0.0, MASKVAL).astype(np.float32)
    mask1 = np.where(j <= 128 + i, 0.0, MASKVAL).astype(np.float32)
    return mask0, mask1


def _in_maps(x, w_q, w_k, w_v):
    import ml_dtypes
    bf = ml_dtypes.bfloat16
    x = np.asarray(x, dtype=np.float32)
    wqT = np.ascontiguousarray(np.asarray(w_q, np.float32).T).astype(bf)
    wkT = np.ascontiguousarray(np.asarray(w_k, np.float32).T).astype(bf)
    wvT = np.ascontiguousarray(np.asarray(w_v, np.float32).T).astype(bf)
    mask0, mask1 = _masks()

    in_maps = []
    for c in range(NCORES):
        b, p = divmod(c, 2)
        xb = x[b]                                    # [S, E]
        xkvT = np.ascontiguousarray(xb[p * HT:(p + 1) * HT].T).astype(bf)
        qrows = np.concatenate(
            [xb[128 * (2 * (k - 1) + p):128 * (2 * (k - 1) + p) + 128, :]
             for k in range(1, NSLOT + 1)], axis=0)  # [NQ, E]
        xqT = np.ascontiguousarray(qrows.T).astype(bf)
        in_maps.append({
            "xkvT": xkvT, "xqT": xqT,
            "wqT": wqT, "wkT": wkT, "wvT": wvT,
            "mask": mask0 if p == 0 else mask1,
        })
    return in_maps


def _scatter(per_core_out):
    out = np.empty((B, S, D), dtype=np.float32)
    for c in range(NCORES):
        b, p = divmod(c, 2)
        oc = per_core_out[c]                         # [NQ, D]
        for k in range(1, NSLOT + 1):
            g = 2 * (k - 1) + p
            out[b, 128 * g:128 * (g + 1), :] = oc[128 * (k - 1):128 * k, :]
    return out


def kernel(x, w_q, w_k, w_v):
    global _prog
    if _prog is None:
        _prog = _build()
    in_maps = _in_maps(x, w_q, w_k, w_v)
    res = run_bass_kernel_spmd(_prog, in_maps, list(range(NCORES)))
    return _scatter([res.results[c]["out"] for c in range(NCORES)])


# revision 10
# speedup vs baseline: 1.1293x; 1.0536x over previous
"""Causal single-head attention on 8 Trainium2 NeuronCores.

Problem: x [4, 2048, 1024], w_q/w_k/w_v [1024, 1024] (nn.Linear convention,
y = x @ W.T). Computes q,k,v projections, causal softmax(q k^T / sqrt(D)) @ v.

Sharding: 2 cores per batch element; query tiles interleaved by parity so
every core has one 128-query tile per kv-length class k (window 256*k).

v3 design (vs the fp32r baseline):
  * K/V projections are computed for HALF the tokens per core (the pair
    core computes the other half) and exchanged with an intra-pair
    AllGather (bf16 payload) — removes the duplicated half of the K/V
    GEMMs (-28% PE work). K and V share the same x tiles (x loaded once).
  * Everything runs in bf16 on the PE (same rate as fp32r, half the DMA
    bytes, much cheaper LDWEIGHTS). fp8 was measured at rel err 6e-2
    (> 2e-2 gate) and rejected; all-bf16 measures ~5e-3.
  * Q^T stays in SBUF (no DRAM spill).
  * PSUM->SBUF P^T copies alternate between ACT and DVE (the slot phase
    is vector-throughput-sensitive).
"""
import numpy as np
from contextlib import ExitStack

import concourse.bass as bass
import concourse.tile as tile
import concourse.mybir as mybir
from concourse.bass_utils import run_bass_kernel_spmd
from concourse.masks import make_identity

F32 = mybir.dt.float32
BF16 = mybir.dt.bfloat16
AF = mybir.ActivationFunctionType
AX = mybir.AxisListType

B, S, E, D = 4, 2048, 1024, 1024
NCORES = 8
NSLOT = 8              # slots k=1..8, kv window = 256*k tokens
NQ = NSLOT * 128       # queries per core
EC = E // 128          # e-chunks
DC = D // 128          # d-chunks
HT = S // 2            # tokens per core half
SCALE = 1.0 / 32.0     # 1/sqrt(D)
MASKVAL = -30000.0

_prog = None


def _split_multi_waits(nc, max_waits=1):
    """The walrus build in this container has one sync-wait slot per
    instruction; hoist extra waits onto preceding same-engine NoOps."""
    n = 0
    for f in nc.m.functions:
        for b in f.blocks:
            insts = b.instructions
            out = []
            changed = False
            for ins in insts:
                si = ins.sync_info
                if si is not None and len(si.on_wait) > max_waits:
                    waits = list(si.on_wait)
                    for w in waits[:-max_waits]:
                        nop = mybir.InstNoOp(name=f"I-waitsplit-{n}")
                        n += 1
                        nop.engine = ins.engine
                        nop.sync_info = mybir.SyncInfo(on_wait=[w], on_update=[])
                        out.append(nop)
                    ins.sync_info = mybir.SyncInfo(
                        on_wait=waits[-max_waits:], on_update=list(si.on_update))
                    changed = True
                out.append(ins)
            if changed:
                b.instructions = out
    return nc


def _build(split=True):
    nc = bass.Bass(trn_type="TRN2", target_bir_lowering=False, debug=False)
    xkvT = nc.dram_tensor("xkvT", [E, HT], BF16, kind="ExternalInput").ap()
    xqT = nc.dram_tensor("xqT", [E, NQ], BF16, kind="ExternalInput").ap()
    wqT = nc.dram_tensor("wqT", [E, D], BF16, kind="ExternalInput").ap()
    wkT = nc.dram_tensor("wkT", [E, D], BF16, kind="ExternalInput").ap()
    wvT = nc.dram_tensor("wvT", [E, D], BF16, kind="ExternalInput").ap()
    maskin = nc.dram_tensor("mask", [128, 256], F32, kind="ExternalInput").ap()
    out = nc.dram_tensor("out", [NQ, D], F32, kind="ExternalOutput").ap()
    # AllGather bounce buffers: my K^T half [dc, 128, HT], my V half
    # [tc, 128, D]; gathered = [member, ...] in absolute token order.
    bK = nc.dram_tensor("bK", [DC, 128, HT], BF16).ap()
    bV = nc.dram_tensor("bV", [HT // 256, 128, D], BF16).ap()
    gK = nc.dram_tensor("gK", [2, DC, 128, HT], BF16).ap()
    bV2 = nc.dram_tensor("bV2", [HT // 256, 128, D], BF16).ap()
    gV2 = nc.dram_tensor("gV2", [2, HT // 256, 128, D], BF16).ap()
    gV = nc.dram_tensor("gV", [2, HT // 256, 128, D], BF16).ap()
    bW = nc.dram_tensor("bW", [128, 4], BF16).ap()
    gW = nc.dram_tensor("gW", [2, 128, 4], BF16).ap()
    RG = [[0, 1], [2, 3], [4, 5], [6, 7]]

    with tile.TileContext(nc) as tc, ExitStack() as ctx:
        # tiny warmup AllGather: absorbs the one-time CC stream startup
        # (~25us) under the input-DMA phase so the real gathers start
        # transferring immediately
        nc.gpsimd.collective_compute(
            "AllGather", mybir.AluOpType.bypass, replica_groups=RG,
            ins=[bW[:]], outs=[gW[:]])
        const = ctx.enter_context(tc.tile_pool(name="const", bufs=1))
        ident = const.tile([128, 128], BF16)
        make_identity(nc, ident[:])
        mask_sb = const.tile([128, 256], F32)
        nc.sync.dma_start(mask_sb[:], maskin[:])

        # persistent attention operands (bf16)
        kvq = ctx.enter_context(tc.tile_pool(name="kvq", bufs=1))
        kts = kvq.tile([128, DC, S], BF16)        # [d-in-chunk, dc, t]
        vts = kvq.tile([128, S // 128, D], BF16)  # [t-in-chunk, tc, d]
        qts = kvq.tile([128, DC, NQ], BF16)       # [d-in-chunk, dc, q]

        # ---- Phase 1: K^T half + V half (shared x tiles), then Q^T ----
        with tc.tile_pool(name="wp", bufs=1) as wp, \
             tc.tile_pool(name="xp", bufs=1) as xp, \
             tc.tile_pool(name="st", bufs=1) as stp, \
             tc.tile_pool(name="pp1", bufs=6, space="PSUM") as pp:
            wk = [wp.tile([128, D], BF16, name=f"wk{e}") for e in range(EC)]
            wv = [wp.tile([128, D], BF16, name=f"wv{e}") for e in range(EC)]
            wq = [wp.tile([128, D], BF16, name=f"wq{e}") for e in range(EC)]
            xgs = [[xp.tile([128, 512], BF16, name=f"x1_{g}_{e}")
                    for e in range(EC)] for g in range(HT // 512)]
            # critical first wave, quarter-split so the first (wk, x)
            # chunks land on parallel DMA queues fast
            for e in range(EC):
                nc.sync.dma_start(wk[e][:, :256],
                                  wkT[e * 128:(e + 1) * 128, :256])
                nc.sync.dma_start(wk[e][:, 256:512],
                                  wkT[e * 128:(e + 1) * 128, 256:512])
                nc.sync.dma_start(xgs[0][e][:, :256],
                                  xkvT[e * 128:(e + 1) * 128, :256])
                nc.sync.dma_start(xgs[0][e][:, 256:],
                                  xkvT[e * 128:(e + 1) * 128, 256:512])
            for e in range(EC):
                nc.sync.dma_start(wk[e][:, 512:],
                                  wkT[e * 128:(e + 1) * 128, 512:])
                nc.sync.dma_start(wv[e][:], wvT[e * 128:(e + 1) * 128, :])
            for e in range(EC):
                nc.sync.dma_start(xgs[1][e][:], xkvT[e * 128:(e + 1) * 128,
                                                     512:1024])
                nc.sync.dma_start(wq[e][:], wqT[e * 128:(e + 1) * 128, :])
            # K and V interleaved per 512-token group (keeps PE/DVE/PSUM
            # pipelined); V AllGather fires per group, K's after group 1
            for g in range(HT // 512):
                xg = xgs[g]
                if g == 0:
                    # e-outer startup: each arriving (wk,x) chunk feeds 4
                    # matmuls immediately
                    for dh in range(2):
                        psl = [pp.tile([128, 512], F32, name=f"pk0_{dh}_{d}",
                                       tag="pp") for d in range(4)]
                        for e in range(EC):
                            for d in range(4):
                                dd = dh * 4 + d
                                nc.tensor.matmul(psl[d][:],
                                                 wk[e][:, dd * 128:(dd + 1) * 128],
                                                 xg[e][:], start=(e == 0),
                                                 stop=(e == EC - 1))
                        for d in range(4):
                            dd = dh * 4 + d
                            ks = stp.tile([128, 512], BF16, name="ks", tag="ks",
                                          bufs=4)
                            nc.vector.tensor_copy(ks[:], psl[d][:])
                            nc.sync.dma_start(bK[dd, :, :512], ks[:])
                else:
                    for d in range(DC):
                        ps = pp.tile([128, 512], F32, name=f"pk{g}_{d}", tag="pp")
                        for e in range(EC):
                            nc.tensor.matmul(ps[:], wk[e][:, d * 128:(d + 1) * 128],
                                             xg[e][:], start=(e == 0),
                                             stop=(e == EC - 1))
                        ks = stp.tile([128, 512], BF16, name="ks", tag="ks", bufs=4)
                        if d % 2 == 0:
                            nc.vector.tensor_copy(ks[:], ps[:])
                        else:
                            nc.scalar.copy(ks[:], ps[:])
                        nc.sync.dma_start(bK[d, :, g * 512:(g + 1) * 512], ks[:])
                # V for this group (same xg tiles; stationary x t-chunks)
                bVg = bV if g == 0 else bV2
                for st in range(4):
                    for dh in range(2):
                        ps = pp.tile([128, 512], F32, name=f"pv{g}_{st}_{dh}",
                                     tag="pp")
                        for e in range(EC):
                            nc.tensor.matmul(ps[:], xg[e][:, st * 128:(st + 1) * 128],
                                             wv[e][:, dh * 512:(dh + 1) * 512],
                                             start=(e == 0), stop=(e == EC - 1))
                        vs = stp.tile([128, 512], BF16, name="vs", tag="vs", bufs=4)
                        if st % 2 == 0:
                            nc.vector.tensor_copy(vs[:], ps[:])
                        else:
                            nc.scalar.copy(vs[:], ps[:])
                        nc.sync.dma_start(bVg[st, :, dh * 512:(dh + 1) * 512],
                                          vs[:])
            # gathers: K first (slots need it first), then V groups
            nc.gpsimd.collective_compute(
                "AllGather", mybir.AluOpType.bypass, replica_groups=RG,
                ins=[bK[:]], outs=[gK[:]])
            for m in range(2):
                nc.sync.dma_start(
                    kts[:, :, m * HT:(m + 1) * HT],
                    gK[m].rearrange("c p t -> p c t"))
            for g in range(HT // 512):
                bVg, gVg = (bV, gV) if g == 0 else (bV2, gV2)
                nc.gpsimd.collective_compute(
                    "AllGather", mybir.AluOpType.bypass, replica_groups=RG,
                    ins=[bVg[:]], outs=[gVg[:]])
                for m in range(2):
                    nc.sync.dma_start(
                        vts[:, m * 8 + g * 4:m * 8 + g * 4 + 4, :],
                        gVg[m].rearrange("c p d -> p c d"))

            # Q^T straight into SBUF (no DRAM spill)
            for g in range(NQ // 512):
                xq = [xp.tile([128, 512], BF16, name=f"x1q_{e}", tag=f"xg{e}",
                              bufs=2) for e in range(EC)]
                for e in range(EC):
                    nc.sync.dma_start(xq[e][:], xqT[e * 128:(e + 1) * 128,
                                                    g * 512:(g + 1) * 512])
                for d in range(DC):
                    ps = pp.tile([128, 512], F32, name=f"pq{g}_{d}", tag="pp")
                    for e in range(EC):
                        nc.tensor.matmul(ps[:], wq[e][:, d * 128:(d + 1) * 128],
                                         xq[e][:], start=(e == 0),
                                         stop=(e == EC - 1))
                    nc.scalar.copy(qts[:, d, g * 512:(g + 1) * 512], ps[:])

        # ---- Phase 2: attention, one slot per kv-length class ----
        # big/small interleave keeps PE fed with the small slot's matmuls
        # while the big slot's softmax chain runs on ACT/DVE
        slot_order = [8, 3, 7, 4, 6, 5, 2, 1]
        with tc.tile_pool(name="att", bufs=1) as ap_, \
             tc.tile_pool(name="ps3", bufs=1, space="PSUM") as pp3:
            for k in slot_order:
                kv = 256 * k
                nch = kv // 128
                ngr = (kv + 511) // 512
                q0 = (k - 1) * 128

                s_ps = [pp3.tile([128, 512], F32, name=f"sps{k}_{g}", tag="sps",
                                 bufs=4) for g in range(ngr)]
                for d in range(DC):
                    lhs = qts[:, d, q0:q0 + 128]
                    for g in range(ngr):
                        w = min(512, kv - g * 512)
                        nc.tensor.matmul(s_ps[g][:, :w], lhs,
                                         kts[:, d, g * 512:g * 512 + w],
                                         start=(d == 0), stop=(d == DC - 1))

                # psum -> sbuf copies (mask folded into the last 256 cols)
                # with per-group running max
                s_sb = ap_.tile([128, 2048], F32, name=f"s{k}", tag="s", bufs=2)
                mparts = ap_.tile([128, 4], F32, name=f"mp{k}", tag="mp", bufs=2)
                lg = ngr - 1
                lw = kv - lg * 512
                for g in range(lg):
                    nc.scalar.copy(s_sb[:, g * 512:(g + 1) * 512], s_ps[g][:])
                if lw == 512:
                    nc.scalar.copy(s_sb[:, kv - 512:kv - 256], s_ps[lg][:, :256])
                    nc.vector.tensor_add(s_sb[:, kv - 256:kv],
                                         s_ps[lg][:, 256:512], mask_sb[:])
                else:
                    nc.vector.tensor_add(s_sb[:, kv - 256:kv],
                                         s_ps[lg][:, :256], mask_sb[:])
                for g in range(ngr):
                    w = min(512, kv - g * 512)
                    nc.vector.reduce_max(mparts[:, g:g + 1],
                                         s_sb[:, g * 512:g * 512 + w], axis=AX.X)

                m = ap_.tile([128, 1], F32, name=f"m{k}", tag="m", bufs=2)
                nc.vector.reduce_max(m[:], mparts[:, :ngr], axis=AX.X)
                negm = ap_.tile([128, 1], F32, name=f"negm{k}", tag="negm", bufs=2)
                nc.scalar.mul(negm[:], m[:], -SCALE)
                # per-group exp so transposes can start before the whole row
                # is exponentiated; per-group sums summed at the end
                p_sb = ap_.tile([128, 2048], BF16, name=f"p{k}", tag="p", bufs=2)
                lparts = ap_.tile([128, 4], F32, name=f"lp{k}", tag="lp", bufs=2)
                for g in range(ngr):
                    w = min(512, kv - g * 512)
                    nc.scalar.activation(p_sb[:, g * 512:g * 512 + w],
                                         s_sb[:, g * 512:g * 512 + w], AF.Exp,
                                         bias=negm[:], scale=SCALE,
                                         accum_out=lparts[:, g:g + 1])
                lsum = ap_.tile([128, 1], F32, name=f"lsum{k}", tag="lsum", bufs=2)
                nc.vector.reduce_sum(lsum[:], lparts[:, :ngr], axis=AX.X)
                linv = ap_.tile([128, 1], F32, name=f"linv{k}", tag="linv", bufs=2)
                nc.vector.reciprocal(linv[:], lsum[:])

                # P^T via PE transpose; copies split across ACT/DVE
                pt = ap_.tile([128, S // 128, 128], BF16, name=f"pt{k}", tag="pt",
                              bufs=2)
                for c in range(nch):
                    tps = pp3.tile([128, 128], BF16, name=f"tp{k}_{c}", tag="tps",
                                   bufs=2)
                    nc.tensor.transpose(tps[:], p_sb[:, c * 128:(c + 1) * 128],
                                        ident[:])
                    if c % 2 == 0:
                        nc.scalar.copy(pt[:, c, :], tps[:])
                    else:
                        nc.vector.tensor_copy(pt[:, c, :], tps[:])

                o_ps = [pp3.tile([128, 512], F32, name=f"op{k}_{h}", tag="ops",
                                 bufs=2) for h in range(2)]
                for c in range(nch):
                    lhs = pt[:, c, :]
                    for h in range(2):
                        nc.tensor.matmul(o_ps[h][:], lhs,
                                         vts[:, c, h * 512:(h + 1) * 512],
                                         start=(c == 0), stop=(c == nch - 1))

                o_sb = ap_.tile([128, D], F32, name=f"o{k}", tag="o", bufs=2)
                for h in range(2):
                    nc.vector.tensor_scalar_mul(o_sb[:, h * 512:(h + 1) * 512],
                                                o_ps[h][:], linv[:])
                for q4 in range(4):
                    nc.sync.dma_start(
                        out[(k - 1) * 128:k * 128, q4 * 256:(q4 + 1) * 256],
                        o_sb[:, q4 * 256:(q4 + 1) * 256])
    if split:
        _split_multi_waits(nc)
    return nc


def _masks():
    j = np.arange(256)[None, :]
    i = np.arange(128)[:, None]
    mask0 = np.where(j <= i, 0.0, MASKVAL).astype(np.float32)
    mask1 = np.where(j <= 128 + i, 0.0, MASKVAL).astype(np.float32)
    return mask0, mask1


def _in_maps(x, w_q, w_k, w_v):
    import ml_dtypes
    bf = ml_dtypes.bfloat16
    x = np.asarray(x, dtype=np.float32)
    wqT = np.ascontiguousarray(np.asarray(w_q, np.float32).T).astype(bf)
    wkT = np.ascontiguousarray(np.asarray(w_k, np.float32).T).astype(bf)
    wvT = np.ascontiguousarray(np.asarray(w_v, np.float32).T).astype(bf)
    mask0, mask1 = _masks()

    in_maps = []
    for c in range(NCORES):
        b, p = divmod(c, 2)
        xb = x[b]                                    # [S, E]
        xkvT = np.ascontiguousarray(xb[p * HT:(p + 1) * HT].T).astype(bf)
        qrows = np.concatenate(
            [xb[128 * (2 * (k - 1) + p):128 * (2 * (k - 1) + p) + 128, :]
             for k in range(1, NSLOT + 1)], axis=0)  # [NQ, E]
        xqT = np.ascontiguousarray(qrows.T).astype(bf)
        in_maps.append({
            "xkvT": xkvT, "xqT": xqT,
            "wqT": wqT, "wkT": wkT, "wvT": wvT,
            "mask": mask0 if p == 0 else mask1,
        })
    return in_maps


def _scatter(per_core_out):
    out = np.empty((B, S, D), dtype=np.float32)
    for c in range(NCORES):
        b, p = divmod(c, 2)
        oc = per_core_out[c]                         # [NQ, D]
        for k in range(1, NSLOT + 1):
            g = 2 * (k - 1) + p
            out[b, 128 * g:128 * (g + 1), :] = oc[128 * (k - 1):128 * k, :]
    return out


def kernel(x, w_q, w_k, w_v):
    global _prog
    if _prog is None:
        _prog = _build()
    in_maps = _in_maps(x, w_q, w_k, w_v)
    res = run_bass_kernel_spmd(_prog, in_maps, list(range(NCORES)))
    return _scatter([res.results[c]["out"] for c in range(NCORES)])
